# revision 32
# baseline (speedup 1.0000x reference)
"""Trainium2 Bass kernel: int8 quantized linear.

Computes, bit-exactly vs the fp32/int32 reference:
    acc   = x_q.int32 @ weight_q.T.int32          # [M, K] @ [K, N] -> [M, N]
    y     = acc.float32 * (scale_x * weight_scale / out_scale)
    y_q   = clip(round_half_even(y), -128, 127).int8

Strategy
--------
Tensor-parallel over N (8 cores x 512 output columns). The PE array has no
int8 matmul, but int8 values are exact in bf16 and every partial sum of
integer-valued products stays far below 2^24, so a bf16 matmul with fp32 PSUM
accumulation reproduces the int32 accumulator exactly. Dequant + round +
clamp + int8 cast run on the vector engine using the +/-1.5*2^23
magic-constant trick for round-half-to-even.

Host side: transpose/pack both operands so all DMAs are contiguous
(contraction dim K on partitions), gather per-core outputs along N.
"""

import os
import numpy as np

M, K, N = 8192, 4096, 4096
NCORES = 8
P = 128                 # partitions
NSH = N // NCORES       # 512 output columns per core
KT = K // P             # 32 k-tiles
MT = M // P             # 64 m-tiles

_MAGIC = 12582912.0     # 1.5 * 2**23: fp32 add/sub rounds to nearest-even int
_BOUND = 131072.0       # pre-round clamp (pow2): keeps magic trick valid for any scale

last_exec_time_ns = None
_prog_cache = {}


def build_program(scale: float, mt: int = MT, kt: int = KT, nsh: int = NSH):
    """Build the per-core Bass/Tile program (SPMD: same program, 8 data shards)."""
    import concourse.bass as bass
    import concourse.mybir as mybir
    from concourse import bacc
    from concourse.tile import TileContext

    dt = mybir.dt
    Alu = mybir.AluOpType

    nc = bacc.Bacc(
        "TRN2",
        target_bir_lowering=False,
        debug=False,
        num_devices=NCORES,
    )
    x = nc.declare_dram_parameter("x", [mt, P, kt * P], dt.bfloat16, isOutput=False)
    w = nc.declare_dram_parameter("w", [P, kt * nsh], dt.bfloat16, isOutput=False)
    y = nc.declare_dram_parameter("y", [mt * P, nsh], dt.int8, isOutput=True)

    with TileContext(nc) as tc:
        with (
            tc.tile_pool(name="wpool", bufs=1) as wpool,
            tc.tile_pool(name="xpool", bufs=6) as xpool,
            tc.tile_pool(name="psum", bufs=6, space=bass.MemorySpace.PSUM) as psum_pool,
            tc.tile_pool(name="warmp", bufs=1, space=bass.MemorySpace.PSUM) as warm_pool,
            tc.tile_pool(name="stage", bufs=3) as stage,
        ):
            wt = wpool.tile([P, kt * nsh], dt.bfloat16)

            # PE pre-warm: the HAM clock gate holds the PE at 1.2GHz until it
            # has been busy ~3.4us. The real stream is now stall-free, so its
            # first ~6 matmuls would pay the cold clock on the critical path;
            # instead, dummy matmuls on zeroed SBUF (no DMA dependencies)
            # warm the PE during the startup DMA window.
            wl = stage.tile([P, P], dt.bfloat16, tag="warml")
            wr = stage.tile([P, nsh], dt.bfloat16, tag="warmr")
            nc.vector.memset(wl[:], 0.0)
            nc.gpsimd.memset(wr[:], 0.0)
            wps = warm_pool.tile([P, nsh], dt.float32)
            for _ in range(16):
                nc.tensor.matmul(wps[:], wl[:], wr[:], start=True, stop=True)

            def mm_group(ps_ap, xt, k0, k1, first, last):
                for ki in range(k0, k1):
                    nc.tensor.matmul(
                        ps_ap,
                        xt[:, ki * P:(ki + 1) * P],
                        wt[:, ki * nsh:(ki + 1) * nsh],
                        start=(first and ki == k0),
                        stop=(last and ki == k1 - 1),
                        skip_group_check=True,
                    )

            def load_x(mi):
                xt = xpool.tile([P, kt * P], dt.bfloat16, tag="xt")
                nc.sync.dma_start(xt[:], x[mi])
                return xt

            def dequant_store(mi, ps_ap, n0, nw, eng=None):
                    eng = eng or nc.vector
                    # One fp32 temp reused in place: elementwise ops are
                    # read-before-write per element, and fewer live tiles
                    # means fewer semaphores for the kernel epilogue to reset.
                    t = stage.tile([P, nw], dt.float32, tag="t1")
                    # t = min(acc * scale, B) -- RN multiply matches reference
                    # (PSUM read must be DVE; GpSimd has no PSUM port)
                    nc.vector.tensor_scalar(
                        out=t[:], in0=ps_ap, scalar1=float(scale),
                        scalar2=_BOUND, op0=Alu.mult, op1=Alu.min,
                    )
                    # t = max(t, -B) + 1.5*2^23  -- RNE to integer
                    eng.tensor_scalar(
                        out=t[:], in0=t[:], scalar1=-_BOUND, scalar2=_MAGIC,
                        op0=Alu.max, op1=Alu.add,
                    )
                    # t = min(t - 1.5*2^23, 127)
                    eng.tensor_scalar(
                        out=t[:], in0=t[:], scalar1=_MAGIC, scalar2=127.0,
                        op0=Alu.subtract, op1=Alu.min,
                    )
                    # y8 = int8(max(t, -128)) -- exact integer, conversion exact
                    y8 = stage.tile([P, nw], dt.int8, tag="y8")
                    eng.tensor_scalar(
                        out=y8[:], in0=t[:], scalar1=-128.0, scalar2=None,
                        op0=Alu.max,
                    )
                    nc.sync.dma_start(y[mi * P:(mi + 1) * P, n0:n0 + nw], y8[:])

            def finish_tile(mi, ps):
                if mi == mt - 1 and nsh % 2 == 0:
                    # Final tile sits on the kernel's critical tail: dequant
                    # the two halves on different engines (DVE reads PSUM for
                    # both, then the chains run on DVE and GpSimd in parallel)
                    dequant_store(mi, ps[:, 0:nsh // 2], 0, nsh // 2, nc.vector)
                    dequant_store(mi, ps[:, nsh // 2:nsh], nsh // 2, nsh // 2, nc.gpsimd)
                else:
                    dequant_store(mi, ps[:], 0, nsh)

            # Startup: only half of w is needed to START accumulating, so run
            # k-half-0 for the first `split` tiles into separate PSUM banks
            # while the second half of w streams in, then resume each bank
            # with k-half-1 (has_written persists per bank, so interleaved
            # accumulation groups are safe). This keeps the PE continuously
            # busy through the 5MB startup DMA window instead of pacing on
            # full-K availability. DMA issue order matches consumption; w
            # quarters stay coarse (bandwidth drops for small descriptors).
            kh = kt // 2
            split = 4 if (mt > 5 and kt % 4 == 0) else 0
            wg = max(1, kt // 4)
            x0 = load_x(0)
            nc.sync.dma_start(wt[:, 0:wg * nsh], w[:, 0:wg * nsh])
            nc.sync.dma_start(wt[:, wg * nsh:kh * nsh], w[:, wg * nsh:kh * nsh])
            xts = [x0] + [load_x(mi) for mi in range(1, split)]
            nc.sync.dma_start(wt[:, kh * nsh:3 * wg * nsh], w[:, kh * nsh:3 * wg * nsh])
            nc.sync.dma_start(wt[:, 3 * wg * nsh:kt * nsh], w[:, 3 * wg * nsh:kt * nsh])

            if split:
                pss = []
                for mi in range(split):
                    ps = psum_pool.tile([P, nsh], dt.float32, tag="ps")
                    mm_group(ps[:], xts[mi], 0, kh, first=True, last=False)
                    pss.append(ps)
                for mi in range(split):
                    mm_group(pss[mi][:], xts[mi], kh, kt, first=False, last=True)
                    finish_tile(mi, pss[mi])

            for mi in range(split, mt):
                xt = xts[mi] if mi < len(xts) else load_x(mi)
                ps = psum_pool.tile([P, nsh], dt.float32, tag="ps")
                mm_group(ps[:], xt, 0, kt, first=True, last=True)
                finish_tile(mi, ps)
    nc.compile()
    return nc


def pack_x(x_q: np.ndarray):
    """[M, K] int8 -> [MT, P, KT*P] bf16 with K on partitions, contiguous DMA."""
    import ml_dtypes
    xp = np.ascontiguousarray(
        x_q.reshape(MT, P, KT, P).transpose(0, 3, 2, 1)
    ).reshape(MT, P, K)
    return xp.astype(ml_dtypes.bfloat16)


def pack_w(weight_q: np.ndarray, core: int):
    """[N, K] int8 row-slice for `core` -> [P, KT*NSH] bf16 (K on partitions)."""
    import ml_dtypes
    wc = weight_q[core * NSH:(core + 1) * NSH, :]          # [NSH, K]
    wp = np.ascontiguousarray(
        wc.reshape(NSH, KT, P).transpose(2, 1, 0)
    ).reshape(P, KT * NSH)
    return wp.astype(ml_dtypes.bfloat16)


def kernel(x_q, scale_x, weight_q, weight_scale, out_scale):
    global last_exec_time_ns
    from concourse.bass_utils import run_bass_kernel_spmd

    x_q = np.asarray(x_q)
    weight_q = np.asarray(weight_q)
    scale = np.float32(scale_x) * np.float32(weight_scale) / np.float32(out_scale)

    key = float(scale)
    nc = _prog_cache.get(key)
    if nc is None:
        nc = _prog_cache[key] = build_program(key)

    xp = pack_x(x_q)
    in_maps = [{"x": xp, "w": pack_w(weight_q, c)} for c in range(NCORES)]

    trace = bool(int(os.environ.get("KERNEL_TRACE", "0")))
    res = run_bass_kernel_spmd(nc, in_maps, list(range(NCORES)), trace=trace)
    last_exec_time_ns = res.exec_time_ns

    y_full = np.concatenate([res.results[c]["y"] for c in range(NCORES)], axis=1)
    return (y_full, np.float32(out_scale))


# revision 33
# speedup vs baseline: 1.0278x; 1.0278x over previous
"""Trainium2 Bass kernel: int8 quantized linear.

Computes, bit-exactly vs the fp32/int32 reference:
    acc   = x_q.int32 @ weight_q.T.int32          # [M, K] @ [K, N] -> [M, N]
    y     = acc.float32 * (scale_x * weight_scale / out_scale)
    y_q   = clip(round_half_even(y), -128, 127).int8

Strategy
--------
Tensor-parallel over N (8 cores x 512 output columns). The PE array has no
int8 matmul, but int8 values are exact in bf16 and every partial sum of
integer-valued products stays far below 2^24, so a bf16 matmul with fp32 PSUM
accumulation reproduces the int32 accumulator exactly. Dequant + round +
clamp + int8 cast run on the vector engine using the +/-1.5*2^23
magic-constant trick for round-half-to-even.

Host side: transpose/pack both operands so all DMAs are contiguous
(contraction dim K on partitions), gather per-core outputs along N.
"""

import os
import numpy as np

M, K, N = 8192, 4096, 4096
NCORES = 8
P = 128                 # partitions
NSH = N // NCORES       # 512 output columns per core
KT = K // P             # 32 k-tiles
MT = M // P             # 64 m-tiles

_MAGIC = 12582912.0     # 1.5 * 2**23: fp32 add/sub rounds to nearest-even int
_BOUND = 131072.0       # pre-round clamp (pow2): keeps magic trick valid for any scale

last_exec_time_ns = None
_prog_cache = {}


def build_program(scale: float, mt: int = MT, kt: int = KT, nsh: int = NSH):
    """Build the per-core Bass/Tile program (SPMD: same program, 8 data shards)."""
    import concourse.bass as bass
    import concourse.mybir as mybir
    from concourse import bacc
    from concourse.tile import TileContext

    dt = mybir.dt
    Alu = mybir.AluOpType

    nc = bacc.Bacc(
        "TRN2",
        target_bir_lowering=False,
        debug=False,
        num_devices=NCORES,
    )
    x = nc.declare_dram_parameter("x", [mt, P, kt * P], dt.bfloat16, isOutput=False)
    w = nc.declare_dram_parameter("w", [P, kt * nsh], dt.bfloat16, isOutput=False)
    y = nc.declare_dram_parameter("y", [mt * P, nsh], dt.int8, isOutput=True)

    with TileContext(nc) as tc:
        with (
            tc.tile_pool(name="wpool", bufs=1) as wpool,
            tc.tile_pool(name="xpool", bufs=6) as xpool,
            tc.tile_pool(name="psum", bufs=6, space=bass.MemorySpace.PSUM) as psum_pool,
            tc.tile_pool(name="stage", bufs=3) as stage,
        ):
            wt = wpool.tile([P, kt * nsh], dt.bfloat16)

            def mm_group(ps_ap, xt, k0, k1, first, last):
                for ki in range(k0, k1):
                    nc.tensor.matmul(
                        ps_ap,
                        xt[:, ki * P:(ki + 1) * P],
                        wt[:, ki * nsh:(ki + 1) * nsh],
                        start=(first and ki == k0),
                        stop=(last and ki == k1 - 1),
                        skip_group_check=True,
                    )

            def load_x(mi):
                xt = xpool.tile([P, kt * P], dt.bfloat16, tag="xt")
                nc.sync.dma_start(xt[:], x[mi])
                return xt

            def dequant_store(mi, ps_ap, n0, nw):
                    # One fp32 temp reused in place: DVE elementwise ops are
                    # read-before-write per element, and fewer live tiles
                    # means fewer semaphores for the kernel epilogue to reset.
                    t = stage.tile([P, nw], dt.float32, tag="t1")
                    # t = min(acc * scale, B) -- RN multiply matches reference
                    nc.vector.tensor_scalar(
                        out=t[:], in0=ps_ap, scalar1=float(scale),
                        scalar2=_BOUND, op0=Alu.mult, op1=Alu.min,
                    )
                    # t = max(t, -B) + 1.5*2^23  -- RNE to integer
                    nc.vector.tensor_scalar(
                        out=t[:], in0=t[:], scalar1=-_BOUND, scalar2=_MAGIC,
                        op0=Alu.max, op1=Alu.add,
                    )
                    # t = min(t - 1.5*2^23, 127)
                    nc.vector.tensor_scalar(
                        out=t[:], in0=t[:], scalar1=_MAGIC, scalar2=127.0,
                        op0=Alu.subtract, op1=Alu.min,
                    )
                    # y8 = int8(max(t, -128)) -- exact integer, conversion exact
                    y8 = stage.tile([P, nw], dt.int8, tag="y8")
                    nc.vector.tensor_scalar(
                        out=y8[:], in0=t[:], scalar1=-128.0, scalar2=None,
                        op0=Alu.max,
                    )
                    nc.sync.dma_start(y[mi * P:(mi + 1) * P, n0:n0 + nw], y8[:])

            def finish_tile(mi, ps):
                if mi == mt - 1 and nsh % 2 == 0:
                    # split the final tile's dequant so its store overlaps the
                    # chain -- this sits on the kernel's critical tail
                    dequant_store(mi, ps[:, 0:nsh // 2], 0, nsh // 2)
                    dequant_store(mi, ps[:, nsh // 2:nsh], nsh // 2, nsh // 2)
                else:
                    dequant_store(mi, ps[:], 0, nsh)

            # Startup: only half of w is needed to START accumulating, so run
            # k-half-0 for the first `split` tiles into separate PSUM banks
            # while the second half of w streams in, then resume each bank
            # with k-half-1 (has_written persists per bank, so interleaved
            # accumulation groups are safe). This keeps the PE continuously
            # busy through the 5MB startup DMA window instead of pacing on
            # full-K availability. DMA issue order matches consumption; w
            # quarters stay coarse (bandwidth drops for small descriptors).
            kh = kt // 2
            split = 4 if (mt > 5 and kt % 4 == 0) else 0
            wg = max(1, kt // 4)
            x0 = load_x(0)
            nc.sync.dma_start(wt[:, 0:wg * nsh], w[:, 0:wg * nsh])
            nc.sync.dma_start(wt[:, wg * nsh:kh * nsh], w[:, wg * nsh:kh * nsh])
            xts = [x0] + [load_x(mi) for mi in range(1, split)]
            nc.sync.dma_start(wt[:, kh * nsh:3 * wg * nsh], w[:, kh * nsh:3 * wg * nsh])
            nc.sync.dma_start(wt[:, 3 * wg * nsh:kt * nsh], w[:, 3 * wg * nsh:kt * nsh])

            if split:
                pss = []
                for mi in range(split):
                    ps = psum_pool.tile([P, nsh], dt.float32, tag="ps")
                    mm_group(ps[:], xts[mi], 0, kh, first=True, last=False)
                    pss.append(ps)
                for mi in range(split):
                    mm_group(pss[mi][:], xts[mi], kh, kt, first=False, last=True)
                    finish_tile(mi, pss[mi])

            for mi in range(split, mt):
                xt = xts[mi] if mi < len(xts) else load_x(mi)
                ps = psum_pool.tile([P, nsh], dt.float32, tag="ps")
                mm_group(ps[:], xt, 0, kt, first=True, last=True)
                finish_tile(mi, ps)
    nc.compile()
    return nc


def pack_x(x_q: np.ndarray):
    """[M, K] int8 -> [MT, P, KT*P] bf16 with K on partitions, contiguous DMA."""
    import ml_dtypes
    xp = np.ascontiguousarray(
        x_q.reshape(MT, P, KT, P).transpose(0, 3, 2, 1)
    ).reshape(MT, P, K)
    return xp.astype(ml_dtypes.bfloat16)


def pack_w(weight_q: np.ndarray, core: int):
    """[N, K] int8 row-slice for `core` -> [P, KT*NSH] bf16 (K on partitions)."""
    import ml_dtypes
    wc = weight_q[core * NSH:(core + 1) * NSH, :]          # [NSH, K]
    wp = np.ascontiguousarray(
        wc.reshape(NSH, KT, P).transpose(2, 1, 0)
    ).reshape(P, KT * NSH)
    return wp.astype(ml_dtypes.bfloat16)


def kernel(x_q, scale_x, weight_q, weight_scale, out_scale):
    global last_exec_time_ns
    from concourse.bass_utils import run_bass_kernel_spmd

    x_q = np.asarray(x_q)
    weight_q = np.asarray(weight_q)
    scale = np.float32(scale_x) * np.float32(weight_scale) / np.float32(out_scale)

    key = float(scale)
    nc = _prog_cache.get(key)
    if nc is None:
        nc = _prog_cache[key] = build_program(key)

    xp = pack_x(x_q)
    in_maps = [{"x": xp, "w": pack_w(weight_q, c)} for c in range(NCORES)]

    trace = bool(int(os.environ.get("KERNEL_TRACE", "0")))
    res = run_bass_kernel_spmd(nc, in_maps, list(range(NCORES)), trace=trace)
    last_exec_time_ns = res.exec_time_ns

    y_full = np.concatenate([res.results[c]["y"] for c in range(NCORES)], axis=1)
    return (y_full, np.float32(out_scale))
